# revision 1
# baseline (speedup 1.0000x reference)
"""Trainium2 Bass kernel (fp16 device compute) for nn_CustomLinear (learned-twiddle butterfly net).

Math (validated vs reference in numpy, rel err ~5e-16):
  reference pads x [2048,4096] to [2048,8192], half-swaps (XOR N/2), then 13
  radix-2 butterfly stages with learned twiddles.  After the half-swap the lo
  half is zero, so the nonzero 4096-vector goes through:
    - stages 1..7  == one 128x128 complex matrix M per 128-block
    - stage  8     == adds only; its twiddle is FOLDED into M for odd
                     blocks (M_od = diag(tw8) @ M)
    - stage  9     == elementwise butterflies (per-partition scalar twiddles)
    - stages 10..12== twiddle product on PE as diagonal matmuls, adds on DVE
    - stage 13     == out = [t, -t], t = c13 * v: folded into the
                     transpose-out matmuls (rhs = per-block diag(c13)); only
                     t is written; the host materializes [t, -t].

Everything on device is bf16 (matmuls accumulate fp32 in PSUM). Correctness
gate is max-normalized rel_err < 2e-2; bf16 lands ~1e-3.

Sharding: pure data parallel, batch 2048 -> 8 cores x 256 rows.
"""
import numpy as np
import ml_dtypes
from contextlib import ExitStack

import concourse.bacc as bacc
import concourse.mybir as mybir
from concourse.tile import TileContext
from concourse.bass_utils import run_bass_kernel_spmd

N = 8192
B = 2048
IN_F = 4096
NCORES = 8
B_CORE = B // NCORES          # 256 rows per core
NTILES = B_CORE // 128        # 2 row-tiles of 128 rows
NBLK = 32                     # nonzero 128-blocks per row
BF = mybir.dt.float16
F32 = mybir.dt.float32
NPBF = np.float16

PE_STAGES = (9, 10, 11, 12)   # stages whose twiddle mult runs on PE

# ---- cwa column layout (ident + M + stage-9..12 twiddle columns) ----
_ID = 0
_MEVR, _MEVI, _MODR, _MODI = 128, 256, 384, 512
_TWR, _TWI = 640, 670         # 30 cols each (stages 9..12, jr-major)
CWA_W = 704

# ---- cwd: diag tiles [Dre | Dim | -Dim] per jr-set; PE stages then st13 ----
_NSET_S = sum(1 << (s - 8) for s in PE_STAGES)
_D13 = _NSET_S * 384
CWD_W = _D13 + NBLK * 512

_CACHE = {}


def _stage_tw(s, w):
    step = 1 << s
    half = step >> 1
    k = np.arange(half) * (N // step)
    ang = (-2.0 * np.pi / N) * k.astype(np.float64) * w[k].astype(np.float64)
    return np.exp(1j * ang)


def _host_consts(w):
    M = np.eye(128, dtype=np.complex128)
    for s in range(1, 8):
        step = 1 << s
        half = step >> 1
        tw = _stage_tw(s, w)
        Bm = np.zeros((step, step), np.complex128)
        Bm[:half, :half] = np.eye(half)
        Bm[:half, half:] = np.diag(tw)
        Bm[half:, :half] = np.eye(half)
        Bm[half:, half:] = -np.diag(tw)
        M = np.kron(np.eye(128 // step), Bm) @ M
    tw8 = _stage_tw(8, w)
    M_od = np.diag(tw8) @ M

    cwa = np.zeros((128, CWA_W), np.float32)
    cwa[:, _ID:_ID + 128] = np.eye(128, dtype=np.float32)
    # lhsT tiles: lhsT[e, e'] = M[e', e]  (out = lhsT.T @ rhs = M @ rhs)
    cwa[:, _MEVR:_MEVR + 128] = M.real.T
    cwa[:, _MEVI:_MEVI + 128] = M.imag.T
    cwa[:, _MODR:_MODR + 128] = M_od.real.T
    cwa[:, _MODI:_MODI + 128] = M_od.imag.T
    off = 0
    for s in range(9, 13):
        tw = _stage_tw(s, w)
        hb = 1 << (s - 8)
        for jr in range(hb):
            cwa[:, _TWR + off] = tw.real[jr * 128:(jr + 1) * 128]
            cwa[:, _TWI + off] = tw.imag[jr * 128:(jr + 1) * 128]
            off += 1

    cwd = np.zeros((128, CWD_W), np.float32)
    ii = np.arange(128)
    off = 0
    for s in PE_STAGES:
        tw = _stage_tw(s, w)
        for jr in range(1 << (s - 8)):
            dre = tw.real[jr * 128:(jr + 1) * 128]
            dim = tw.imag[jr * 128:(jr + 1) * 128]
            cwd[ii, off + ii] = dre
            cwd[ii, off + 128 + ii] = dim
            cwd[ii, off + 256 + ii] = -dim
            off += 384
    c13 = _stage_tw(13, w)
    for j in range(NBLK):
        dre = c13.real[j * 128:(j + 1) * 128]
        dim = c13.imag[j * 128:(j + 1) * 128]
        # paired rhs: [Dre | Dim] then [-Dim | Dre]
        cwd[ii, off + ii] = dre
        cwd[ii, off + 128 + ii] = dim
        cwd[ii, off + 256 + ii] = -dim
        cwd[ii, off + 384 + ii] = dre
        off += 512
    return cwa.astype(NPBF), cwd.astype(NPBF)


def _dset_off(s):
    off = 0
    for t in PE_STAGES:
        if t == s:
            return off
        off += (1 << (t - 8)) * 384
    raise ValueError(s)


def _build_program():
    nc = bacc.Bacc("TRN2", target_bir_lowering=False, debug=False)
    x_d = nc.dram_tensor("x", [B_CORE, IN_F], BF, kind="ExternalInput").ap()
    cwa_d = nc.dram_tensor("cwa", [128, CWA_W], BF, kind="ExternalInput").ap()
    cwd_d = nc.dram_tensor("cwd", [128, CWD_W], BF, kind="ExternalInput").ap()
    y_d = nc.dram_tensor("y", [B_CORE, 2 * IN_F], BF, kind="ExternalOutput").ap()

    AL = mybir.AluOpType

    with TileContext(nc) as tc, ExitStack() as ctx:
        cpool = ctx.enter_context(tc.tile_pool(name="const", bufs=1))
        xpool = ctx.enter_context(tc.tile_pool(name="xin", bufs=1))
        xtpool = ctx.enter_context(tc.tile_pool(name="xt", bufs=1))
        zpool = ctx.enter_context(tc.tile_pool(name="z", bufs=2))
        tpool = ctx.enter_context(tc.tile_pool(name="t", bufs=1))
        opool = ctx.enter_context(tc.tile_pool(name="out", bufs=4))
        ps = ctx.enter_context(tc.tile_pool(name="ps", bufs=4, space="PSUM"))

        cwa = cpool.tile([128, CWA_W], BF)
        nc.sync.dma_start(cwa[:], cwa_d[:])
        cwd = cpool.tile([128, CWD_W], BF)
        xts = []
        for ti in range(NTILES):
            # XBAR DMA-transpose: per-128-block transposed load from DRAM
            xt = xtpool.tile([128, IN_F], BF, tag=f"xt{ti}")
            xv3 = xt[:].rearrange("p (j r) -> p j r", j=NBLK)
            nc.sync.dma_start_transpose(xv3, x_d[ti * 128:ti * 128 + 128, :])
            xts.append(xt)
        nc.sync.dma_start(cwd[:, :_D13], cwd_d[:, :_D13])
        nc.sync.dma_start(cwd[:, _D13:], cwd_d[:, _D13:])
        ident = cwa[:, _ID:_ID + 128]
        mevr = cwa[:, _MEVR:_MEVR + 128]
        mevi = cwa[:, _MEVI:_MEVI + 128]
        modr = cwa[:, _MODR:_MODR + 128]
        modi = cwa[:, _MODI:_MODI + 128]

        def dset(s, j):
            o = _dset_off(s) + j * 384
            return (cwd[:, o:o + 128],
                    cwd[:, o + 128:o + 256],
                    cwd[:, o + 256:o + 384])

        def dset13(j):
            o = _D13 + j * 512
            return cwd[:, o:o + 256], cwd[:, o + 256:o + 512]

        def make_tile(ti):
            """Phase closures for one 128-row tile; emitted per the schedule."""
            st = {}
            r0 = ti * 128

            def pa():
                xt = xts[ti]
                zre = zpool.tile([128, IN_F], BF, tag="zre")
                zim = zpool.tile([128, IN_F], BF, tag="zim")
                st["zre"], st["zim"] = zre, zim
                xv = xt[:].rearrange("p (g c) -> p g c", g=16)
                zvr = zre[:].rearrange("p (g c) -> p g c", g=16)
                zvi = zim[:].rearrange("p (g c) -> p g c", g=16)
                t8r = tpool.tile([128, 2048], BF, tag=f"t8r{ti}")
                t8i = tpool.tile([128, 2048], BF, tag=f"t8i{ti}")
                t8rv = t8r[:].rearrange("p (g c) -> p g c", g=16)
                t8iv = t8i[:].rearrange("p (g c) -> p g c", g=16)
                for c in range(4):                   # chunks of 4 pairs
                    g0, g1 = c * 4, c * 4 + 4
                    ev = xv[:, g0:g1, 0:128]
                    od = xv[:, g0:g1, 128:256]
                    p_evr = ps.tile([128, 512], F32, tag=f"pm{ti}")
                    p_evi = ps.tile([128, 512], F32, tag=f"pm{ti}")
                    p_odr = ps.tile([128, 512], F32, tag=f"pm{ti}")
                    p_odi = ps.tile([128, 512], F32, tag=f"pm{ti}")
                    nc.tensor.matmul(p_evr[:], mevr, ev, start=True, stop=True)
                    nc.tensor.matmul(p_evi[:], mevi, ev, start=True, stop=True)
                    nc.tensor.matmul(p_odr[:], modr, od, start=True, stop=True)
                    nc.tensor.matmul(p_odi[:], modi, od, start=True, stop=True)
                    nc.scalar.copy(zvr[:, g0:g1, 0:128], p_evr[:])
                    nc.scalar.copy(zvi[:, g0:g1, 0:128], p_evi[:])
                    nc.vector.tensor_copy(t8rv[:, g0:g1, :], p_odr[:])
                    nc.scalar.copy(t8iv[:, g0:g1, :], p_odi[:])
                # stage 8 adds (twiddle folded into M_od)
                nc.vector.tensor_tensor(zvr[:, :, 128:256], zvr[:, :, 0:128],
                                        t8rv[:, :, :], op=AL.subtract)
                nc.gpsimd.tensor_tensor(zvi[:, :, 128:256], zvi[:, :, 0:128],
                                        t8iv[:, :, :], op=AL.subtract)
                nc.vector.tensor_tensor(zvr[:, :, 0:128], zvr[:, :, 0:128],
                                        t8rv[:, :, :], op=AL.add)
                nc.vector.tensor_tensor(zvi[:, :, 0:128], zvi[:, :, 0:128],
                                        t8iv[:, :, :], op=AL.add)

            def stage(s):
                zre, zim = st["zre"], st["zim"]
                G = 1 << (s - 7)
                hb = G // 2
                ng = NBLK // G
                z4r = zre[:].rearrange("p (g j e) -> p g j e", g=ng, j=G)
                z4i = zim[:].rearrange("p (g j e) -> p g j e", g=ng, j=G)
                tr = tpool.tile([128, 2048], BF, tag=f"str{ti}")
                ti_ = tpool.tile([128, 2048], BF, tag=f"sti{ti}")
                t3r = tr[:].rearrange("p (j g e) -> p g j e", j=hb, g=ng)
                t3i = ti_[:].rearrange("p (j g e) -> p g j e", j=hb, g=ng)
                w_ = ng * 128
                for u in range(4):
                    p_tr = ps.tile([128, 512], F32, tag=f"pm{ti}")
                    p_ti = ps.tile([128, 512], F32, tag=f"pm{ti}")
                    if s == 9:                       # unit = (jr, g-half)
                        jr, h = u // 2, u % 2
                        dre, dim, mdim = dset(s, jr)
                        hr = z4r[:, h * 4:h * 4 + 4, hb + jr, :]
                        hi = z4i[:, h * 4:h * 4 + 4, hb + jr, :]
                        nc.tensor.matmul(p_tr[:], dre, hr,
                                         start=True, stop=False)
                        nc.tensor.matmul(p_tr[:], mdim, hi,
                                         start=False, stop=True)
                        nc.tensor.matmul(p_ti[:], dim, hr,
                                         start=True, stop=False)
                        nc.tensor.matmul(p_ti[:], dre, hi,
                                         start=False, stop=True)
                        o0 = jr * 1024 + h * 512
                    else:                            # unit = 512/w_ jr values
                        per = 512 // w_
                        for k in range(per):
                            jr = u * per + k
                            dre, dim, mdim = dset(s, jr)
                            hr = z4r[:, :, hb + jr, :]
                            hi = z4i[:, :, hb + jr, :]
                            sr = p_tr[:, k * w_:(k + 1) * w_]
                            si = p_ti[:, k * w_:(k + 1) * w_]
                            nc.tensor.matmul(sr, dre, hr,
                                             start=True, stop=False)
                            nc.tensor.matmul(sr, mdim, hi,
                                             start=False, stop=True)
                            nc.tensor.matmul(si, dim, hr,
                                             start=True, stop=False)
                            nc.tensor.matmul(si, dre, hi,
                                             start=False, stop=True)
                        o0 = u * 512
                    if u % 2 == 0:
                        nc.scalar.copy(tr[:, o0:o0 + 512], p_tr[:])
                        nc.vector.tensor_copy(ti_[:, o0:o0 + 512], p_ti[:])
                    else:
                        nc.vector.tensor_copy(tr[:, o0:o0 + 512], p_tr[:])
                        nc.scalar.copy(ti_[:, o0:o0 + 512], p_ti[:])
                # batched adds: hi' = lo - t first, then lo' += t
                lo_r, lo_i = z4r[:, :, 0:hb, :], z4i[:, :, 0:hb, :]
                hi_r, hi_i = z4r[:, :, hb:G, :], z4i[:, :, hb:G, :]
                nc.vector.tensor_tensor(hi_r, lo_r, t3r, op=AL.subtract)
                eng_hi_i = nc.gpsimd if s in (9, 10, 12) else nc.vector
                eng_hi_i.tensor_tensor(hi_i, lo_i, t3i, op=AL.subtract)
                nc.vector.tensor_tensor(lo_r, lo_r, t3r, op=AL.add)
                nc.vector.tensor_tensor(lo_i, lo_i, t3i, op=AL.add)

            def out():
                zre, zim = st["zre"], st["zim"]
                # psum per 2 blocks: [pr_b(128)|pi_b(128)] x2, block-major
                for jc in range(NBLK // 4):
                    pa_ = ps.tile([128, 512], F32, tag=f"pm{ti}")
                    pb_ = ps.tile([128, 512], F32, tag=f"pm{ti}")
                    for k in range(4):
                        j = jc * 4 + k
                        bs = slice(j * 128, (j + 1) * 128)
                        d1, d2 = dset13(j)
                        tgt = (pa_ if k < 2 else pb_)[
                            :, (k % 2) * 256:(k % 2) * 256 + 256]
                        nc.tensor.matmul(tgt, zre[:, bs], d1,
                                         start=True, stop=False)
                        nc.tensor.matmul(tgt, zim[:, bs], d2,
                                         start=False, stop=True)
                    op_ = opool.tile([128, 1024], BF, tag="op")
                    if jc % 2 == 0:
                        nc.scalar.copy(op_[:, 0:512], pa_[:])
                        nc.vector.tensor_copy(op_[:, 512:1024], pb_[:])
                    else:
                        nc.vector.tensor_copy(op_[:, 0:512], pa_[:])
                        nc.scalar.copy(op_[:, 512:1024], pb_[:])
                    c0 = jc * 1024
                    nc.sync.dma_start(y_d[r0:r0 + 128, c0:c0 + 1024], op_[:])

            return {"pa": pa, "stage": stage, "out": out}

        t0 = make_tile(0)
        t1 = make_tile(1)
        # software-pipelined emission: tile-1 phases fill tile-0 stalls
        t0["pa"]()
        t0["stage"](9)
        t1["pa"]()
        t0["stage"](10)
        t1["stage"](9)
        t0["stage"](11)
        t1["stage"](10)
        t0["stage"](12)
        t1["stage"](11)
        t0["out"]()
        t1["stage"](12)
        t1["out"]()

    nc.compile()
    return nc


def kernel(x: np.ndarray, weights: np.ndarray) -> np.ndarray:
    x = np.asarray(x, dtype=np.float32)
    w = np.asarray(weights, dtype=np.float32)
    xb = np.ascontiguousarray(x.astype(NPBF))
    if "nc" not in _CACHE:
        _CACHE["nc"] = _build_program()
    nc = _CACHE["nc"]
    cwa, cwd = _host_consts(w)
    in_maps = [
        {"x": xb[ci * B_CORE:(ci + 1) * B_CORE], "cwa": cwa, "cwd": cwd}
        for ci in range(NCORES)
    ]
    res = run_bass_kernel_spmd(nc, in_maps, list(range(NCORES)))
    _CACHE["last_results"] = res
    t = np.concatenate([res.results[ci]["y"] for ci in range(NCORES)], axis=0)
    # y row layout: 32 x [re_block(128) | im_block(128)], block-major
    t = t.astype(np.float32).reshape(B, NBLK, 2, 128)
    t = (t[:, :, 0, :] + 1j * t[:, :, 1, :]).astype(np.complex64)
    t = t.reshape(B, IN_F)
    return np.concatenate([t, -t], axis=1)           # [2048, 8192]

